# revision 1
# baseline (speedup 1.0000x reference)
"""Trainium2 Bass kernel for nn_Column (nms_detection).

Computation (matches the reference exactly):
  out[t,k]  = sum_chw rec_field[t,chw] * weight[k,chw]        (32x512 <- contract 100000)
  pot       = out * (out > 10) ; spike = (out > 10)
  nspk[k]   = sum_t spike ; first[k] = min(32 - nspk, 31)
  values[k] = pot[first[k], k] ; v = max_k(values * (nspk>0)) * 32
  total     = nspk*values + nspk*v
  coef      = top-16 nonzero mask of total (== sequential argmax-suppress set)
  result    = spike * coef[broadcast]                          (32x512 of 0.0/1.0)

Distribution: contraction dim (100000) sharded 8 ways (12500 rows/core, padded
to 12544 = 98*128).  Matmul runs in bf16 hi/lo split form (exactly the
decomposition the HW fp32 path uses internally, so fp32 precision): stationary
[x_hi | x_lo] (128,64) per chunk, moving w_hi / w_lo passes accumulating a
(64,Ws) PSUM folded after the loop.  The K=512 output columns are processed in
three splits (256/128/128) so the first two 32/16KB AllReduces overlap the
remaining DMA+matmul work and most per-split epilogue pre-work hides under the
later collectives; only the last 16KB AllReduce (~10us warm) plus the small
k-WTA join is exposed.  Every core redundantly computes the k-WTA epilogue;
core 0's output is returned.
"""

import numpy as np
import ml_dtypes

import concourse.bacc as bacc
import concourse.mybir as mybir
from concourse.tile import TileContext
from concourse.bass_utils import run_bass_kernel_spmd

T = 32               # timesteps
K = 512              # out_channels / features
CTOT = 100000        # in_channels * rf_size * length (1*50*2000)
NCORES = 8
SH = CTOT // NCORES  # 12500 contraction rows per core
NCH = 98             # 128-row contraction chunks per core
SHP = NCH * 128      # 12544 (zero padded)
GROUP = 7            # chunks per W DMA group
NG = NCH // GROUP    # 14 groups per split
SPLITS = [(0, 320), (320, 512)]
XCUTS = [(0, 2), (2, 16), (16, NCH)]  # X DMA pieces (chunk ranges)
THRESH = 10.0
F32 = mybir.dt.float32
BF16 = mybir.dt.bfloat16
NPBF16 = ml_dtypes.bfloat16

_CACHE = {}


def _build_nc():
    nc = bacc.Bacc("TRN2", target_bir_lowering=False, debug=False, num_devices=NCORES)

    # x: per chunk c the stationary block [x_hi | x_lo] (128,64) bf16
    x_d = nc.dram_tensor("x", [128, NCH * 2 * T], BF16, kind="ExternalInput")
    # w: split-major; per split s, chunk c: [w_hi (128,Ws) | w_lo (128,Ws)]
    w_d = nc.dram_tensor("w", [128, 2 * NCH * K], BF16, kind="ExternalInput")
    oc_d = nc.dram_tensor("onescol", [T, 1], F32, kind="ExternalInput")
    o32_d = nc.dram_tensor("ones32", [T, T], F32, kind="ExternalInput")
    or_d = nc.dram_tensor("onesrow", [1, T], BF16, kind="ExternalInput")
    tp_d = nc.dram_tensor("tpos32", [T, 1], F32, kind="ExternalInput")
    out_d = nc.dram_tensor("out", [T, K], F32, kind="ExternalOutput")

    nsp = len(SPLITS)
    widths = [b - a for a, b in SPLITS]

    with TileContext(nc) as tc:
        with (
            tc.tile_pool(name="sb", bufs=1) as sb,
            tc.tile_pool(name="wp0", bufs=3) as wp0,
            tc.tile_pool(name="wp12", bufs=4) as wp12,
            tc.tile_pool(name="ps", bufs=1, space="PSUM") as ps,
            tc.tile_pool(name="dram", bufs=1, space="DRAM") as dr,
        ):
            # X in three pieces (so the first chunks land early), consts, then
            # W groups alternating between the sync and scalar HWDGE rings.
            xsb = sb.tile([128, NCH * 2 * T], BF16)
            for a, b in XCUTS:
                nc.sync.dma_start(xsb[:, a * 2 * T:b * 2 * T],
                                  x_d[:, a * 2 * T:b * 2 * T])
            oc = sb.tile([T, 1], F32)
            nc.sync.dma_start(oc[:], oc_d[:])
            o32 = sb.tile([T, T], F32)
            nc.sync.dma_start(o32[:], o32_d[:])
            orr = sb.tile([1, T], BF16)
            nc.sync.dma_start(orr[:], or_d[:])
            tpos = sb.tile([T, 1], F32)
            nc.sync.dma_start(tpos[:], tp_d[:])

            # full-bank (2KB/partition) accum tiles: no PSUM bank sharing
            accum_full = [ps.tile([2 * T, 512], F32, name=f"accum{s}")
                          for s, w in enumerate(widths)]
            accum = [accum_full[s][:, 0:w] for s, w in enumerate(widths)]
            part = [sb.tile([T, w], F32, name=f"part{s}")
                    for s, w in enumerate(widths)]
            bins = [dr.tile([T, w], F32, name=f"bin{s}")
                    for s, w in enumerate(widths)]
            bouts = [dr.tile([T, w], F32, addr_space="Shared", name=f"bout{s}")
                     for s, w in enumerate(widths)]
            lo64 = sb.tile([2 * T, K], F32)  # staging for the hi/lo folds

            woff = 0
            wbase = []
            for s, w in enumerate(widths):
                wbase.append(woff)
                woff += NCH * 2 * w

            def emit_split(s):
                a, b = SPLITS[s]
                w = widths[s]
                pool = wp0 if s == 0 else wp12
                for g in range(NG):
                    wt = pool.tile([128, GROUP * 2 * w], BF16, tag=f"wt{min(s,1)}")
                    # X rides on sync first, so scalar takes split-0's first
                    # two groups back-to-back to avoid an early stall.
                    if s == 0:
                        eng = nc.scalar if (g < 2 or g % 2 == 1) else nc.sync
                    else:
                        eng = nc.sync if (g % 2 == 1) else nc.scalar
                    base = wbase[s] + g * GROUP * 2 * w
                    eng.dma_start(wt[:], w_d[:, base:base + GROUP * 2 * w])
                    for c in range(GROUP):
                        cc = g * GROUP + c
                        xst = xsb[:, cc * 2 * T:(cc + 1) * 2 * T]
                        nc.tensor.matmul(
                            accum[s][:], xst, wt[:, c * 2 * w:c * 2 * w + w],
                            start=(cc == 0), stop=False,
                        )
                        nc.tensor.matmul(
                            accum[s][:], xst, wt[:, c * 2 * w + w:(c + 1) * 2 * w],
                            start=False, stop=(cc == NCH - 1),
                        )
                # fold hi/lo rows: part[s] = accum[s][0:32] + accum[s][32:64].
                # PSUM feeds at most one DVE input and DMA can't read PSUM:
                # copy lo rows to SBUF, move down to partitions 0-31 (gpsimd
                # software DMA -- the HW rings stay on W), then add.
                lo = lo64[:, a:b]
                nc.vector.tensor_copy(lo[T:2 * T, :], accum[s][T:2 * T, :])
                nc.gpsimd.dma_start(lo[0:T, :], lo[T:2 * T, :])
                nc.vector.tensor_tensor(part[s][:], accum[s][0:T, :], lo[0:T, :],
                                        mybir.AluOpType.add)
                nc.gpsimd.dma_start(bins[s][:], part[s][:])
                nc.gpsimd.collective_compute(
                    "AllReduce",
                    mybir.AluOpType.add,
                    replica_groups=[list(range(NCORES))],
                    ins=[bins[s].opt()],
                    outs=[bouts[s].opt()],
                )

            # per-split epilogue tiles (contiguous; strided slices of a full
            # (32,512) tile are ~3x slower on the DVE)
            ofull = [sb.tile([T, w], F32, name=f"ofull{s}")
                     for s, w in enumerate(widths)]
            spike = [sb.tile([T, w], F32, name=f"spike{s}")
                     for s, w in enumerate(widths)]
            pot = [sb.tile([T, w], F32, name=f"pot{s}")
                   for s, w in enumerate(widths)]
            pv = [sb.tile([T, w], F32, name=f"pv{s}")
                  for s, w in enumerate(widths)]
            # reuse the (dead after the fold) accum PSUM for nspkb / coefb
            nspkb = [accum[s][0:T, :] for s in range(nsp)]
            vals_ps = ps.tile([1, K], F32)
            nrow = sb.tile([1, K], F32)

            def emit_prework(s):
                a, b = SPLITS[s]
                # gpsimd queue order makes this wait for AllReduce s results
                nc.gpsimd.dma_start(ofull[s][:], bouts[s][:])
                nc.vector.tensor_scalar(spike[s][:], ofull[s][:], THRESH, None,
                                        op0=mybir.AluOpType.is_gt)
                # pot = (ofull > 10) * ofull in one fused op
                nc.vector.scalar_tensor_tensor(pot[s][:], ofull[s][:], THRESH,
                                               ofull[s][:],
                                               op0=mybir.AluOpType.is_gt,
                                               op1=mybir.AluOpType.mult)
                # nspk broadcast to all 32 rows: ones(32,32).T @ spike
                nc.tensor.matmul(nspkb[s][:], o32[:], spike[s][:],
                                 start=True, stop=True)
                # onehot(nspk == 32-t) * pot ; values[k] = sum_t of that
                nc.vector.scalar_tensor_tensor(pv[s][:], nspkb[s][:], tpos[:],
                                               pot[s][:],
                                               op0=mybir.AluOpType.is_equal,
                                               op1=mybir.AluOpType.mult)
                nc.tensor.matmul(vals_ps[0:1, a:b], oc[:], pv[s][:],
                                 start=True, stop=True)
                nc.scalar.copy(nrow[0:1, a:b], nspkb[s][0:1, :])

            # all splits (and their AllReduce triggers) go first on the
            # gpsimd queue; the ofull readbacks in the preworks come after so
            # an AllReduce-completion wait never delays a later AR trigger.
            for s in range(nsp):
                emit_split(s)
            for s in range(nsp):
                emit_prework(s)

            # ---- k-WTA join (only this part is after the last AllReduce) ----
            # v*32 = max(values) * 32  (values[k] is 0 exactly when nspk==0)
            vmax = sb.tile([1, 1], F32)
            nc.vector.tensor_reduce(vmax[:], vals_ps[:], axis=mybir.AxisListType.X,
                                    op=mybir.AluOpType.max)
            vmax32 = sb.tile([1, 1], F32)
            nc.vector.tensor_scalar(vmax32[:], vmax[:], float(T), None,
                                    op0=mybir.AluOpType.mult)
            # total = (values + vmax32) * nspk   (one fused op)
            total = sb.tile([1, K], F32)
            nc.vector.scalar_tensor_tensor(total[:], vals_ps[:], vmax32[:],
                                           nrow[:],
                                           op0=mybir.AluOpType.add,
                                           op1=mybir.AluOpType.mult)

            # top-16 nonzero mask: two rounds of (8-max, match-replace-with-0).
            # Zero entries "win" as no-ops and never enter the mask, matching
            # the reference's invalid-winner (-1) behavior.
            work = sb.tile([1, K], F32)
            s8a = sb.tile([1, 8], F32)
            nc.vector.max(s8a[:], total[:])
            nc.vector.match_replace(work[:], s8a[:], total[:], 0.0)
            s8b = sb.tile([1, 8], F32)
            nc.vector.max(s8b[:], work[:])
            nc.vector.match_replace(work[:], s8b[:], work[:], 0.0)

            # winner mask as bf16 0/1 (exact), broadcast via bf16 matmul
            cmask = sb.tile([1, K], BF16)
            nc.vector.tensor_tensor(cmask[:], total[:], work[:],
                                    mybir.AluOpType.is_gt)
            for s, (a, b) in enumerate(SPLITS):
                w = widths[s]
                coefb = accum[s][0:T, :]  # reuse PSUM again (nspkb consumed)
                nc.tensor.matmul(coefb[:], orr[:], cmask[0:1, a:b],
                                 start=True, stop=True)
                res = sb.tile([T, w], F32, name=f"res{s}")
                nc.vector.scalar_tensor_tensor(res[:], coefb[:], 0.0, spike[s][:],
                                               op0=mybir.AluOpType.is_gt,
                                               op1=mybir.AluOpType.mult)
                eng = nc.sync if s % 2 == 0 else nc.scalar
                eng.dma_start(out_d[:, a:b], res[:])

    nc.compile()
    return nc


def _get_nc():
    if "nc" not in _CACHE:
        _CACHE["nc"] = _build_nc()
    return _CACHE["nc"]


def _split_bf16(a):
    """Split fp32 array into (hi, lo) bf16 parts: hi + lo == a to ~2^-18 rel."""
    hi = a.astype(NPBF16)
    lo = (a - hi.astype(np.float32)).astype(NPBF16)
    return hi, lo


def _pack_inputs(rec_field, weight):
    X = np.ascontiguousarray(np.asarray(rec_field, dtype=np.float32).reshape(T, CTOT))
    W = np.ascontiguousarray(np.asarray(weight, dtype=np.float32).reshape(K, CTOT))
    oc = np.ones((T, 1), np.float32)
    o32 = np.ones((T, T), np.float32)
    orr = np.ones((1, T), NPBF16)
    tp = (float(T) - np.arange(T, dtype=np.float32)).reshape(T, 1)
    in_maps = []
    for i in range(NCORES):
        xp = np.zeros((T, SHP), np.float32)
        xp[:, :SH] = X[:, i * SH:(i + 1) * SH]
        wp = np.zeros((K, SHP), np.float32)
        wp[:, :SH] = W[:, i * SH:(i + 1) * SH]
        # (contract, n) -> chunks (NCH,128,n)
        xpc = xp.T.reshape(NCH, 128, T)
        wpc = wp.T.reshape(NCH, 128, K)
        xh, xl = _split_bf16(xpc)
        wh, wl = _split_bf16(wpc)
        # per chunk stationary [x_hi | x_lo]: (NCH,128,2T) -> (128, NCH*2T)
        xpk = np.ascontiguousarray(
            np.concatenate([xh, xl], axis=2).transpose(1, 0, 2).reshape(128, NCH * 2 * T))
        # w: split-major layout; per split s, chunk c: [w_hi | w_lo] (Ws each)
        parts = []
        for a, b in SPLITS:
            blk = np.concatenate([wh[:, :, a:b], wl[:, :, a:b]], axis=2)
            parts.append(blk.transpose(1, 0, 2).reshape(128, -1))
        wpk = np.ascontiguousarray(np.concatenate(parts, axis=1))
        in_maps.append({"x": xpk, "w": wpk, "onescol": oc, "ones32": o32,
                        "onesrow": orr, "tpos32": tp})
    return in_maps


def kernel(rec_field, weight, _trace=False, _trace_kwargs=None):
    nc = _get_nc()
    in_maps = _pack_inputs(rec_field, weight)
    r = run_bass_kernel_spmd(nc, in_maps, list(range(NCORES)), trace=_trace,
                             **(_trace_kwargs or {}))
    _CACHE["last_results"] = r
    out = np.asarray(r.results[0]["out"], dtype=np.float32)
    return out.reshape(T, K, 1, 1)



# revision 2
# speedup vs baseline: 1.2171x; 1.2171x over previous
"""Trainium2 Bass kernel for nn_Column (nms_detection).

Computation (matches the reference exactly):
  out[t,k]  = sum_chw rec_field[t,chw] * weight[k,chw]        (32x512 <- contract 100000)
  pot       = out * (out > 10) ; spike = (out > 10)
  nspk[k]   = sum_t spike ; first[k] = min(32 - nspk, 31)
  values[k] = pot[first[k], k] ; v = max_k(values) * 32
  total     = nspk*values + nspk*v
  coef      = top-16 nonzero mask of total (== sequential argmax-suppress set)
  result    = spike * coef[broadcast]                          (32x512 of 0.0/1.0)

Distribution: contraction dim (100000) sharded 8 ways (12500 rows/core, padded
to 12544 = 98*128).  Per chunk of 128 contraction rows the stationary is
[x_hi | x_lo] (128,64) bf16 and the moving operand is [w_hi | w_lo] (128,1024):
two 512-wide matmuls accumulate x*(w_hi+w_lo) into PSUM, which together with
the hi/lo x rows gives full fp32 precision.  The weight stream (25.7MB/core)
runs on both HWDGE queues with 8KB/partition descriptors at ~330GB/s.

Cross-core reduction avoids the collectives engine entirely (its per-execution
bootstrap barrier costs ~90us): after the fold packs each core's partial into
S (128,128) = (4 t-blocks x 32t, 128 feat), every core remote-DMA-broadcasts S
to all 8 cores (XOR slot scheme, 8 single-dest broadcasts, one trigger).  A
raw (non-Tile) wait_ge on the remote semaphore bridges into a second Tile
context where each core redundantly sums the 8 slots and runs the full k-WTA
epilogue in the packed layout; core 0's (128,128) result is unpacked on host.
"""

import numpy as np
import ml_dtypes

import concourse.bacc as bacc
import concourse.mybir as mybir
from concourse.tile import TileContext
from concourse.bass_utils import run_bass_kernel_spmd

T = 32               # timesteps
K = 512              # out_channels / features
CTOT = 100000        # in_channels * rf_size * length (1*50*2000)
NCORES = 8
SH = CTOT // NCORES  # 12500 contraction rows per core
NCH = 98             # 128-row contraction chunks per core
SHP = NCH * 128      # 12544 (zero padded)
GROUP = 4            # chunks per W DMA group (8KB/partition descriptors)
THRESH = 10.0
F32 = mybir.dt.float32
BF16 = mybir.dt.bfloat16
NPBF16 = ml_dtypes.bfloat16

_CACHE = {}


def _build_nc():
    nc = bacc.Bacc("TRN2", target_bir_lowering=False, debug=False, num_devices=NCORES)

    # x: per chunk c the stationary block [x_hi | x_lo] (128,64) bf16
    x_d = nc.dram_tensor("x", [128, NCH * 2 * T], BF16, kind="ExternalInput")
    # w: chunk-major; per chunk c: [w_hi (128,512) | w_lo (128,512)]
    w_d = nc.dram_tensor("w", [128, NCH * 2 * K], BF16, kind="ExternalInput")
    # consts: tposG (128,1) f32 = 32 - (p%32); bdiag (128,128) bf16 block-diag;
    # bq f32 (128,4) block cols; bqb bf16 (128,4); bmask bf16 (1,512) block rows
    tp_d = nc.dram_tensor("tposg", [128, 1], F32, kind="ExternalInput")
    bd_d = nc.dram_tensor("bdiag", [128, 128], BF16, kind="ExternalInput")
    bq_d = nc.dram_tensor("bqf", [128, 4], F32, kind="ExternalInput")
    bqb_d = nc.dram_tensor("bqb", [128, 4], BF16, kind="ExternalInput")
    bm_d = nc.dram_tensor("bmask", [1, 512], BF16, kind="ExternalInput")
    out_d = nc.dram_tensor("out", [128, 128], F32, kind="ExternalOutput")

    # persistent SBUF (survives both tile contexts): exchange send/recv buffers
    S_t = nc.alloc_sbuf_tensor("S_send", [128, 128], F32)
    recv_t = nc.alloc_sbuf_tensor("recv", [128, NCORES * 128], F32)
    rsem = nc.alloc_semaphore(name="xc_remote")
    lsem = nc.alloc_semaphore(name="xc_local")

    # ---------------- phase A: stream + matmul + fold + send ----------------
    with TileContext(nc) as tc:
        with (
            tc.tile_pool(name="sb", bufs=1) as sb,
            tc.tile_pool(name="wp", bufs=6) as wp,
            tc.tile_pool(name="ps", bufs=1, space="PSUM") as ps,
        ):
            xsb = sb.tile([128, NCH * 2 * T], BF16)
            # first chunks early so matmul can start; rest in one big dma
            nc.gpsimd.dma_start(xsb[:, 0:GROUP * 2 * T], x_d[:, 0:GROUP * 2 * T])
            nc.gpsimd.dma_start(xsb[:, GROUP * 2 * T:], x_d[:, GROUP * 2 * T:])

            acc = [ps.tile([2 * T, 512], F32, name=f"acc{i}") for i in range(2)]

            # weight stream: groups of GROUP chunks, alternating HWDGE queues
            bounds = list(range(0, NCH, GROUP)) + [NCH]
            for gi in range(len(bounds) - 1):
                c0, c1 = bounds[gi], bounds[gi + 1]
                wt = wp.tile([128, (c1 - c0) * 2 * K], BF16, tag="wt")
                eng = nc.sync if gi % 2 == 0 else nc.scalar
                eng.dma_start(wt[:], w_d[:, c0 * 2 * K:c1 * 2 * K])
                for c in range(c0, c1):
                    xst = xsb[:, c * 2 * T:(c + 1) * 2 * T]
                    wof = (c - c0) * 2 * K
                    b = c % 2
                    # both hi and lo moving passes accumulate into the same
                    # bank: acc = x * (w_hi + w_lo) in fp32 PSUM
                    nc.tensor.matmul(acc[b][:], xst, wt[:, wof:wof + K],
                                     start=(c < 2), stop=False)
                    nc.tensor.matmul(acc[b][:], xst, wt[:, wof + K:wof + 2 * K],
                                     start=False, stop=(c >= NCH - 2))

            # fold into packed S (128,128): S[32q+t, c] = sum of 4 acc terms
            # of out[t, 128q+c] (acc{0,1} x rows{hi(x_hi part),lo(x_lo part)})
            S = S_t.ap()
            for q in range(4):
                dst = S[32 * q:32 * q + 32, :]
                nc.vector.tensor_copy(dst, acc[0][0:T, 128 * q:128 * q + 128])
                nc.vector.tensor_tensor(dst, dst, acc[0][T:2 * T, 128 * q:128 * q + 128],
                                        mybir.AluOpType.add)
                nc.vector.tensor_tensor(dst, dst, acc[1][0:T, 128 * q:128 * q + 128],
                                        mybir.AluOpType.add)
                nc.vector.tensor_tensor(dst, dst, acc[1][T:2 * T, 128 * q:128 * q + 128],
                                        mybir.AluOpType.add)

            # broadcast S to every core's recv slot k (slot k on core r holds
            # S of core r^k, possibly ^2-permuted within 4..7 -- sum-invariant)
            recv = recv_t.ap()
            for k in range(NCORES):
                rdests = [None] * NCORES
                rdests[k] = (0, k)
                nc.gpsimd.remote_dma_broadcast(
                    recv[:, 128 * k:128 * (k + 1)], S[:],
                    remote_sem=rsem, local_sem=lsem, rdests=rdests)
            nc.gpsimd.trigger_dma(count=None)

    # raw cross-context wait: every core's slice has landed (8 senders x 2)
    nc.vector.wait_ge(rsem, 2 * NCORES)

    # ---------------- phase B: reduce + epilogue (every core) ----------------
    with TileContext(nc) as tc:
        with (
            tc.tile_pool(name="sb2", bufs=1) as sb,
            tc.tile_pool(name="ps2", bufs=1, space="PSUM") as ps,
        ):
            tpos = sb.tile([128, 1], F32)
            nc.sync.dma_start(tpos[:], tp_d[:])
            bdiag = sb.tile([128, 128], BF16)
            nc.sync.dma_start(bdiag[:], bd_d[:])
            bqf = sb.tile([128, 4], F32)
            nc.sync.dma_start(bqf[:], bq_d[:])
            bqb = sb.tile([128, 4], BF16)
            nc.sync.dma_start(bqb[:], bqb_d[:])
            bmask = sb.tile([1, 512], BF16)
            nc.sync.dma_start(bmask[:], bm_d[:])

            recv = recv_t.ap()
            # tree-sum the 8 slots -> G (128,128) = full out in packed layout
            h0 = sb.tile([128, 128], F32)
            h1 = sb.tile([128, 128], F32)
            nc.vector.tensor_tensor(h0[:], recv[:, 0:128], recv[:, 128:256],
                                    mybir.AluOpType.add)
            nc.vector.tensor_tensor(h0[:], h0[:], recv[:, 256:384],
                                    mybir.AluOpType.add)
            nc.vector.tensor_tensor(h0[:], h0[:], recv[:, 384:512],
                                    mybir.AluOpType.add)
            nc.vector.tensor_tensor(h1[:], recv[:, 512:640], recv[:, 640:768],
                                    mybir.AluOpType.add)
            nc.vector.tensor_tensor(h1[:], h1[:], recv[:, 768:896],
                                    mybir.AluOpType.add)
            nc.vector.tensor_tensor(h1[:], h1[:], recv[:, 896:1024],
                                    mybir.AluOpType.add)
            G = sb.tile([128, 128], F32)
            nc.vector.tensor_tensor(G[:], h0[:], h1[:], mybir.AluOpType.add)

            # spike (bf16 0/1) and pot
            spikeG = sb.tile([128, 128], BF16)
            nc.vector.tensor_scalar(spikeG[:], G[:], THRESH, None,
                                    op0=mybir.AluOpType.is_gt)
            potG = sb.tile([128, 128], F32)
            nc.vector.scalar_tensor_tensor(potG[:], G[:], THRESH, G[:],
                                           op0=mybir.AluOpType.is_gt,
                                           op1=mybir.AluOpType.mult)

            # nspk broadcast within each 32-row block: bdiag.T @ spikeG
            nspkb = ps.tile([128, 128], F32, name="nspkb")
            nc.tensor.matmul(nspkb[:], bdiag[:], spikeG[:], start=True, stop=True)

            # pv = (nspkb == tpos) * pot ; values/nspk rows via block-col matmuls
            pvG = sb.tile([128, 128], F32)
            nc.vector.scalar_tensor_tensor(pvG[:], nspkb[:], tpos[:], potG[:],
                                           op0=mybir.AluOpType.is_equal,
                                           op1=mybir.AluOpType.mult)
            vals_ps = ps.tile([1, 512], F32, name="vals")
            nspk_ps = ps.tile([1, 512], F32, name="nspkr")
            for q in range(4):
                nc.tensor.matmul(vals_ps[0:1, 128 * q:128 * (q + 1)],
                                 bqf[:, q:q + 1], pvG[:], start=True, stop=True)
                nc.tensor.matmul(nspk_ps[0:1, 128 * q:128 * (q + 1)],
                                 bqb[:, q:q + 1], spikeG[:], start=True, stop=True)

            # v*32 = max(values) * 32  (values[k] is 0 exactly when nspk==0)
            vmax = sb.tile([1, 1], F32)
            nc.vector.tensor_reduce(vmax[:], vals_ps[:], axis=mybir.AxisListType.X,
                                    op=mybir.AluOpType.max)
            vmax32 = sb.tile([1, 1], F32)
            nc.vector.tensor_scalar(vmax32[:], vmax[:], float(T), None,
                                    op0=mybir.AluOpType.mult)
            nrow = sb.tile([1, 512], F32)
            nc.scalar.copy(nrow[:], nspk_ps[:])
            # total = (values + vmax32) * nspk
            total = sb.tile([1, 512], F32)
            nc.vector.scalar_tensor_tensor(total[:], vals_ps[:], vmax32[:], nrow[:],
                                           op0=mybir.AluOpType.add,
                                           op1=mybir.AluOpType.mult)

            # top-16 nonzero mask: two rounds of (8-max, match-replace-with-0)
            work = sb.tile([1, 512], F32)
            s8a = sb.tile([1, 8], F32)
            nc.vector.max(s8a[:], total[:])
            nc.vector.match_replace(work[:], s8a[:], total[:], 0.0)
            s8b = sb.tile([1, 8], F32)
            nc.vector.max(s8b[:], work[:])
            nc.vector.match_replace(work[:], s8b[:], work[:], 0.0)
            cmask = sb.tile([1, 512], BF16)
            nc.vector.tensor_tensor(cmask[:], total[:], work[:],
                                    mybir.AluOpType.is_gt)

            # coef in packed layout: 4 block-row broadcasts of cmask slices
            coefG = ps.tile([128, 128], F32, name="coefg")
            for q in range(4):
                nc.tensor.matmul(coefG[:], bmask[0:1, 128 * q:128 * (q + 1)],
                                 cmask[0:1, 128 * q:128 * (q + 1)],
                                 start=(q == 0), stop=(q == 3))
            resG = sb.tile([128, 128], F32)
            nc.vector.scalar_tensor_tensor(resG[:], coefG[:], 0.0, spikeG[:],
                                           op0=mybir.AluOpType.is_gt,
                                           op1=mybir.AluOpType.mult)
            nc.sync.dma_start(out_d[:], resG[:])

    # clear exchange semaphores so back-to-back executions stay correct
    # (on vector so the clears are ordered after the raw wait above)
    nc.vector.sem_clear(rsem)
    nc.vector.sem_clear(lsem)

    nc.compile()
    return nc


def _get_nc():
    if "nc" not in _CACHE:
        _CACHE["nc"] = _build_nc()
    return _CACHE["nc"]


def _split_bf16(a):
    """Split fp32 array into (hi, lo) bf16 parts: hi + lo == a to ~2^-18 rel."""
    hi = a.astype(NPBF16)
    lo = (a - hi.astype(np.float32)).astype(NPBF16)
    return hi, lo


def _consts():
    p = np.arange(128)
    tpos = (float(T) - (p % 32)).astype(np.float32).reshape(128, 1)
    bdiag = ((p[:, None] // 32) == (p[None, :] // 32)).astype(NPBF16)
    bqf = ((p[:, None] // 32) == np.arange(4)[None, :]).astype(np.float32)
    bqb = bqf.astype(NPBF16)
    bmask = ((np.arange(512)[None, :] // 128) ==
             (np.arange(512)[None, :] // 128)).astype(NPBF16)  # placeholder
    # bmask[0, f] used as stationary (1,128) slices; for block q the stationary
    # must be block-membership of the OUTPUT partition m: bmask_q[m] = (m//32==q)
    bm = np.zeros((1, 512), dtype=NPBF16)
    for q in range(4):
        seg = np.zeros(128, np.float32)
        seg[32 * q:32 * (q + 1)] = 1.0
        bm[0, 128 * q:128 * (q + 1)] = seg.astype(NPBF16)
    return tpos, bdiag, bqf, bqb, bm


def _pack_inputs(rec_field, weight):
    X = np.ascontiguousarray(np.asarray(rec_field, dtype=np.float32).reshape(T, CTOT))
    W = np.ascontiguousarray(np.asarray(weight, dtype=np.float32).reshape(K, CTOT))
    tpos, bdiag, bqf, bqb, bm = _consts()
    in_maps = []
    for i in range(NCORES):
        xp = np.zeros((T, SHP), np.float32)
        xp[:, :SH] = X[:, i * SH:(i + 1) * SH]
        wp = np.zeros((K, SHP), np.float32)
        wp[:, :SH] = W[:, i * SH:(i + 1) * SH]
        # (contract, n) -> chunks (NCH,128,n)
        xpc = xp.T.reshape(NCH, 128, T)
        wpc = wp.T.reshape(NCH, 128, K)
        xh, xl = _split_bf16(xpc)
        wh, wl = _split_bf16(wpc)
        # per chunk stationary [x_hi | x_lo]: (NCH,128,2T) -> (128, NCH*2T)
        xpk = np.ascontiguousarray(
            np.concatenate([xh, xl], axis=2).transpose(1, 0, 2).reshape(128, NCH * 2 * T))
        # w chunk-major: per chunk [w_hi | w_lo]: (NCH,128,2K) -> (128, NCH*2K)
        wpk = np.ascontiguousarray(
            np.concatenate([wh, wl], axis=2).transpose(1, 0, 2).reshape(128, NCH * 2 * K))
        in_maps.append({"x": xpk, "w": wpk, "tposg": tpos, "bdiag": bdiag,
                        "bqf": bqf, "bqb": bqb, "bmask": bm})
    return in_maps


def kernel(rec_field, weight, _trace=False, _trace_kwargs=None):
    nc = _get_nc()
    in_maps = _pack_inputs(rec_field, weight)
    r = run_bass_kernel_spmd(nc, in_maps, list(range(NCORES)), trace=_trace,
                             **(_trace_kwargs or {}))
    _CACHE["last_results"] = r
    res = np.asarray(r.results[0]["out"], dtype=np.float32)  # (128,128) packed
    out = res.reshape(4, 32, 128).transpose(1, 0, 2).reshape(T, K)
    return np.ascontiguousarray(out).reshape(T, K, 1, 1)
